# revision 16
# baseline (speedup 1.0000x reference)
"""Fused attention-with-offset kernel for Trainium2, 8-core data-parallel.

Problem (per batch element b, B=8 elements -> one NeuronCore each):
    q = query @ Wq                [SQ, D]
    k = key @ Wk                  [SKV, D]
    v = value @ Wv                [SKV, D]
    scores = (q @ k^T) / sqrt(D)  [SQ, SKV]
    attn = softmax(scores) + offset @ Woff
    out = attn @ v                [SQ, D]

Kernel strategy (all matmuls bf16 on PE, f32 PSUM accumulation):
  - offset path uses associativity: (offset@Woff)@v == offset@(Woff@v),
    cutting its FLOPs from 8.6 GF to 2.4 GF per core.
  - softmax computed unnormalized: expT[t,s] = exp(scale*scores[s,t]) in
    the [kv, q] orientation, row-sums via an extra N=1 matmul against a
    ones vector (lands [q, 1] per-partition), normalization fused into
    the epilogue as a per-partition tensor_scalar multiply.
  - activations enter SBUF transposed ([din, seq]) via a bf16 DRAM stage
    (SWDGE cast-DMA) + XBAR transpose-DMA.
"""

import os
import sys

import numpy as np

sys.path.insert(0, "/opt/trn_rl_repo")
sys.path.insert(0, "/opt/pypackages")

B, SQ, SKV, DIN, DOUT = 8, 2048, 2048, 512, 512
P = 128
SCALE = 1.0 / float(np.sqrt(DOUT))
N_CORES = 8

_CACHED = {}


def _build_bass():
    import concourse.bass as bass
    import concourse.tile as tile
    from concourse import bacc, mybir

    f32 = mybir.dt.float32
    bf16 = mybir.dt.bfloat16
    f8 = mybir.dt.float8e4
    DR = mybir.MatmulPerfMode.DoubleRow
    ts = bass.ts

    nc = bacc.Bacc(
        "TRN2",
        target_bir_lowering=False,
        debug=False,
        enable_asserts=True,
        num_devices=N_CORES,
    )

    query = nc.dram_tensor("query", [SQ, DIN], f32, kind="ExternalInput").ap()
    key = nc.dram_tensor("key", [SKV, DIN], f32, kind="ExternalInput").ap()
    value = nc.dram_tensor("value", [SKV, DIN], f32, kind="ExternalInput").ap()
    offset = nc.dram_tensor("offset", [SQ, DIN], f32, kind="ExternalInput").ap()
    Wq = nc.dram_tensor("Wq", [DIN, DOUT], f32, kind="ExternalInput").ap()
    Wk = nc.dram_tensor("Wk", [DIN, DOUT], f32, kind="ExternalInput").ap()
    Wv = nc.dram_tensor("Wv", [DIN, DOUT], f32, kind="ExternalInput").ap()
    Woff = nc.dram_tensor("Woff", [DIN, SKV], f32, kind="ExternalInput").ap()
    out = nc.dram_tensor("out", [SQ, DOUT], f32, kind="ExternalOutput").ap()

    KI = DIN // P    # 4  din tiles
    MO = DOUT // P   # 4  dout tiles
    TQ = SQ // P     # 16 q tiles
    TK = SKV // P    # 16 kv tiles
    NQ = SQ // 512   # 4  q chunks of 512

    with tile.TileContext(nc) as tc:
        with (
            tc.tile_pool(name="dram", bufs=1, space="DRAM") as dram,
            tc.tile_pool(name="wts", bufs=1) as wts,
            tc.tile_pool(name="actT", bufs=3) as actT,
            tc.tile_pool(name="proj", bufs=1) as proj,
            tc.tile_pool(name="expp", bufs=1) as expp,
            tc.tile_pool(name="eps", bufs=3) as eps,
            tc.tile_pool(name="psum", bufs=4, space="PSUM") as psum,
            tc.tile_pool(name="pstp", bufs=2, space="PSUM") as pstp,
            tc.tile_pool(name="psrs", bufs=2, space="PSUM") as psrs,
        ):
            import concourse.mybir as mybir
            from concourse.masks import make_identity

            # ---- natural cast-loads for query/key (PE-transposed below) ----
            # All plain copies run before any XBAR transpose (Tile serializes
            # every copy<->transpose xbar-mode transition at ~10-25us each),
            # so query/key are transposed on the PE instead: natural bf16
            # cast-load, then tensor-engine transpose-mode.  This lets the PE
            # start ~15us in rather than waiting ~70us for staged casts.
            def load_nat(src, rows, tag):
                nat = actT.tile([P, rows // P, DIN], bf16, tag="actT")
                v3 = src.rearrange("(so p) d -> p so d", p=P)
                for r in range(4):
                    nc.gpsimd.dma_start(nat[:, ts(r, rows // P // 4), :],
                                        v3[:, ts(r, rows // P // 4), :])
                return nat

            qnat = load_nat(query, SQ, "q")

            # identity via NEFF-embedded constant: keeps the gpsimd queue
            # free for the SWDGE cast descgens at kernel start
            import ml_dtypes as _mld
            ident_dram = nc.inline_tensor(
                np.eye(P, dtype=_mld.bfloat16), name="ident_const"
            )
            ident = wts.tile([P, P], bf16, tag="ident")
            nc.sync.dma_start(ident[:], ident_dram.ap())
            # HAM warmup: PE transpose-mode doesn't count as PE-busy for the
            # clock gate, so burn ~3us of dummy matmuls while the first input
            # chunks are still in flight; M1 then starts at 2.4GHz.
            warm = pstp.tile([P, P], f32, tag="pst")
            for i in range(28):
                nc.tensor.matmul(warm[:], lhsT=ident[:], rhs=ident[:],
                                 start=(i == 0), stop=(i == 27))
            # DoubleRow rhs needs dim-1 step % 16 == 0, so pad to [P, 2, 16]
            ones_sb = wts.tile([P, 2, 16], f8, tag="ones")
            nc.vector.memset(ones_sb[:], 1.0)
            expbias = wts.tile([P, 1], f32, tag="expbias")
            nc.vector.memset(expbias[:], -3.0)

            # ---- weights: cast-DMA straight into SBUF ----------------------
            wq_sb = wts.tile([P, KI, DOUT], f8, tag="wq")
            wk_sb = wts.tile([P, KI, DOUT], f8, tag="wk")
            wv_sb = wts.tile([P, KI, DOUT], bf16, tag="wv")
            nc.gpsimd.dma_start(wq_sb[:], Wq.rearrange("(ko p) n -> p ko n", p=P))
            nc.gpsimd.dma_start(wk_sb[:], Wk.rearrange("(ko p) n -> p ko n", p=P))
            nc.gpsimd.dma_start(wv_sb[:], Wv.rearrange("(ko p) n -> p ko n", p=P))

            knat = load_nat(key, SKV, "k")

            # ---- staged loads for value/offset/Woff (XBAR-transposed) ------
            stg_v = dram.tile([SKV, DIN], bf16, tag="stg_v")
            nc.gpsimd.dma_start(stg_v[:], value)
            stg_off = dram.tile([SQ, DIN], bf16, tag="stg_off")
            nc.gpsimd.dma_start(stg_off[:], offset)
            stg_woff = dram.tile([DIN, SKV], bf16, tag="stg_woff")
            nc.gpsimd.dma_start(stg_woff[:], Woff)

            # PE transpose: nat [128(seq), so, din] -> t [128(din), c, seq]
            def pe_transpose(nat, rows, ident):
                t = actT.tile([P, KI, rows], f8, tag="actT")
                for g in range(rows // 512):
                    for c in range(KI):
                        pt = pstp.tile([P, 512], bf16, tag="pst")
                        for j in range(4):
                            nc.tensor.transpose(
                                pt[:, ts(j, P)], nat[:, g * 4 + j, ts(c, P)],
                                ident,
                            )
                        nc.scalar.copy(t[:, c, ts(g, 512)], pt[:])
                return t

            # M1/M2: projections, transposed outputs [dout, seq]
            qpT = proj.tile([P, MO, SQ], f8, tag="qpT")
            kpT = proj.tile([P, MO, SKV], f8, tag="kpT")
            qT = pe_transpose(qnat, SQ, ident)
            kT = pe_transpose(knat, SKV, ident)
            for w_sb, xT, oT, NN in ((wq_sb, qT, qpT, NQ), (wk_sb, kT, kpT, NQ)):
                for m in range(MO):
                    for n in range(NN):
                        pt = psum.tile([P, 512], f32, tag="mm")
                        for k in range(KI // 2):
                            nc.tensor.matmul(
                                pt[:],
                                lhsT=w_sb[:, 2 * k : 2 * k + 2, ts(m, P)],
                                rhs=xT[:, 2 * k : 2 * k + 2, ts(n, 512)],
                                start=(k == 0),
                                stop=(k == KI // 2 - 1),
                                perf_mode=DR,
                            )
                        nc.vector.tensor_copy(oT[:, m, ts(n, 512)], pt[:])

            # XBAR transposes (single copy->transpose mode transition)
            vT = actT.tile([P, KI, SKV], bf16, tag="actT")
            for c in range(KI):
                nc.sync.dma_start_transpose(vT[:, c, :], stg_v[:, ts(c, P)])
            woffT = actT.tile([P, TK, DIN], bf16, tag="actT")
            for c in range(TK):
                nc.sync.dma_start_transpose(woffT[:, c, :], stg_woff[:, ts(c, P)])
            offT = actT.tile([P, KI, SQ], bf16, tag="actT")
            for c in range(KI):
                nc.sync.dma_start_transpose(offT[:, c, :], stg_off[:, ts(c, P)])

            # M4: scoresT [kv, q] -> exp(scale*x) -> bf16 SBUF
            expT = expp.tile([P, TK, SQ], f8, tag="expT")
            for mk in range(TK):
                for n in range(NQ):
                    pt = psum.tile([P, 512], f32, tag="mm")
                    for k in range(MO // 2):
                        nc.tensor.matmul(
                            pt[:],
                            lhsT=kpT[:, 2 * k : 2 * k + 2, ts(mk, P)],
                            rhs=qpT[:, 2 * k : 2 * k + 2, ts(n, 512)],
                            start=(k == 0),
                            stop=(k == MO // 2 - 1),
                            perf_mode=DR,
                        )
                    # bias -3 keeps exp outputs well inside fp8e4 range
                    # (max score*scale ~ 5.5); it cancels exactly in the
                    # rowsum normalization.
                    nc.scalar.activation(
                        expT[:, mk, ts(n, 512)],
                        pt[:],
                        mybir.ActivationFunctionType.Exp,
                        scale=SCALE,
                        bias=expbias[:],
                    )

            # M3: v_proj [kv, dout]  (after M4 in PE order: vT arrives while
            # M4 is running)
            vp = proj.tile([P, TK, DOUT], bf16, tag="vp")
            vp8 = proj.tile([P, TK, DOUT], f8, tag="vp8")
            for mk in range(TK):
                pt = psum.tile([P, 512], f32, tag="mm")
                for k in range(KI):
                    nc.tensor.matmul(
                        pt[:],
                        lhsT=vT[:, k, ts(mk, P)],
                        rhs=wv_sb[:, k, :],
                        start=(k == 0),
                        stop=(k == KI - 1),
                    )
                nc.vector.tensor_copy(vp[:, mk, :], pt[:])
                nc.vector.tensor_copy(vp8[:, mk, :], pt[:])

            # W3' = Woff @ v_proj   [din, dout]
            w3 = wts.tile([P, KI, DOUT], bf16, tag="w3")
            for m in range(KI):
                pt = psum.tile([P, 512], f32, tag="mm")
                for kk in range(TK):
                    nc.tensor.matmul(
                        pt[:],
                        lhsT=woffT[:, kk, ts(m, P)],
                        rhs=vp[:, kk, :],
                        start=(kk == 0),
                        stop=(kk == TK - 1),
                    )
                nc.vector.tensor_copy(w3[:, m, :], pt[:])

            # M5 + rowsum + M7 + epilogue, per q tile
            for mq in range(TQ):
                po = psum.tile([P, 512], f32, tag="mm")
                prs = psrs.tile([P, 1], f32, tag="rs")
                for kk in range(TK // 2):
                    nc.tensor.matmul(
                        po[:],
                        lhsT=expT[:, 2 * kk : 2 * kk + 2, ts(mq, P)],
                        rhs=vp8[:, 2 * kk : 2 * kk + 2, :],
                        start=(kk == 0),
                        stop=(kk == TK // 2 - 1),
                        perf_mode=DR,
                    )
                    nc.tensor.matmul(
                        prs[:],
                        lhsT=expT[:, 2 * kk : 2 * kk + 2, ts(mq, P)],
                        rhs=ones_sb[:, :, :1],
                        start=(kk == 0),
                        stop=(kk == TK // 2 - 1),
                        perf_mode=DR,
                    )
                poff = psum.tile([P, 512], f32, tag="mm")
                for k in range(KI):
                    nc.tensor.matmul(
                        poff[:],
                        lhsT=offT[:, k, ts(mq, P)],
                        rhs=w3[:, k, :],
                        start=(k == 0),
                        stop=(k == KI - 1),
                    )
                rc = eps.tile([P, 1], f32, tag="rc")
                nc.vector.reciprocal(rc[:], prs[:])
                tmp = eps.tile([P, 512], f32, tag="tmp")
                nc.vector.tensor_scalar_mul(tmp[:], po[:], rc[:])
                ot = eps.tile([P, 512], f32, tag="ot")
                nc.vector.tensor_add(ot[:], tmp[:], poff[:])
                nc.sync.dma_start(out[ts(mq, P), :], ot[:])

    nc.compile()
    return nc


def _get_nc():
    if "nc" not in _CACHED:
        _CACHED["nc"] = _build_bass()
    return _CACHED["nc"]


def kernel(**inputs):
    from concourse.bass_utils import run_bass_kernel_spmd

    nc = _get_nc()

    def f32c(x):
        return np.ascontiguousarray(np.asarray(x), dtype=np.float32)

    shared = {k: f32c(inputs[k]) for k in ("Wq", "Wk", "Wv", "Woff")}
    in_maps = [
        {
            "query": f32c(inputs["query"][c]),
            "key": f32c(inputs["key"][c]),
            "value": f32c(inputs["value"][c]),
            "offset": f32c(inputs["offset"][c]),
            **shared,
        }
        for c in range(N_CORES)
    ]
    res = run_bass_kernel_spmd(nc, in_maps, list(range(N_CORES)))
    return np.stack([res.results[c]["out"] for c in range(N_CORES)], axis=0)


def _install_ntff_shim():
    """The agent image's antenv lacks axon_hooks; recreate it so
    run_bass_kernel_spmd(trace=True) can reach the NTFF profiler."""
    import sys as _sys
    import types

    if "antenv.axon_hooks" in _sys.modules:
        return
    mod = types.ModuleType("antenv.axon_hooks")
    _state = {"hook": None}
    mod.set_axon_ntff_profile_hook = lambda h: _state.__setitem__("hook", h)
    mod.get_axon_ntff_profile_hook = lambda: _state["hook"]
    _sys.modules["antenv.axon_hooks"] = mod
    try:
        from trn_agent_boot.trn_boot import _ntff_profile_via_ctypes

        mod.set_axon_ntff_profile_hook(
            _ntff_profile_via_ctypes("/opt/axon/libaxon_pjrt.so")
        )
    except Exception as e:
        print(f"ntff shim: could not install profile hook: {e}", file=sys.stderr)


def run_traced(**inputs):
    """Like kernel(), but also returns (output, exec_time_ns) via NTFF trace."""
    _install_ntff_shim()
    from concourse.bass_utils import run_bass_kernel_spmd

    nc = _get_nc()

    def f32c(x):
        return np.ascontiguousarray(np.asarray(x), dtype=np.float32)

    shared = {k: f32c(inputs[k]) for k in ("Wq", "Wk", "Wv", "Woff")}
    in_maps = [
        {
            "query": f32c(inputs["query"][c]),
            "key": f32c(inputs["key"][c]),
            "value": f32c(inputs["value"][c]),
            "offset": f32c(inputs["offset"][c]),
            **shared,
        }
        for c in range(N_CORES)
    ]
    res = run_bass_kernel_spmd(nc, in_maps, list(range(N_CORES)), trace=True)
    outv = np.stack([res.results[c]["out"] for c in range(N_CORES)], axis=0)
    return outv, res


# revision 17
# speedup vs baseline: 1.0446x; 1.0446x over previous
"""Fused attention-with-offset kernel for Trainium2, 8-core data-parallel.

Problem (per batch element b, B=8 elements -> one NeuronCore each):
    q = query @ Wq                [SQ, D]
    k = key @ Wk                  [SKV, D]
    v = value @ Wv                [SKV, D]
    scores = (q @ k^T) / sqrt(D)  [SQ, SKV]
    attn = softmax(scores) + offset @ Woff
    out = attn @ v                [SQ, D]

Kernel strategy (all matmuls bf16 on PE, f32 PSUM accumulation):
  - offset path uses associativity: (offset@Woff)@v == offset@(Woff@v),
    cutting its FLOPs from 8.6 GF to 2.4 GF per core.
  - softmax computed unnormalized: expT[t,s] = exp(scale*scores[s,t]) in
    the [kv, q] orientation, row-sums via an extra N=1 matmul against a
    ones vector (lands [q, 1] per-partition), normalization fused into
    the epilogue as a per-partition tensor_scalar multiply.
  - activations enter SBUF transposed ([din, seq]) via a bf16 DRAM stage
    (SWDGE cast-DMA) + XBAR transpose-DMA.
"""

import os
import sys

import numpy as np

sys.path.insert(0, "/opt/trn_rl_repo")
sys.path.insert(0, "/opt/pypackages")

B, SQ, SKV, DIN, DOUT = 8, 2048, 2048, 512, 512
P = 128
SCALE = 1.0 / float(np.sqrt(DOUT))
N_CORES = 8

_CACHED = {}


def _build_bass():
    import concourse.bass as bass
    import concourse.tile as tile
    from concourse import bacc, mybir

    f32 = mybir.dt.float32
    bf16 = mybir.dt.bfloat16
    f8 = mybir.dt.float8e4
    DR = mybir.MatmulPerfMode.DoubleRow
    ts = bass.ts

    nc = bacc.Bacc(
        "TRN2",
        target_bir_lowering=False,
        debug=False,
        enable_asserts=True,
        num_devices=N_CORES,
    )

    query = nc.dram_tensor("query", [SQ, DIN], f32, kind="ExternalInput").ap()
    key = nc.dram_tensor("key", [SKV, DIN], f32, kind="ExternalInput").ap()
    value = nc.dram_tensor("value", [SKV, DIN], f32, kind="ExternalInput").ap()
    offset = nc.dram_tensor("offset", [SQ, DIN], f32, kind="ExternalInput").ap()
    Wq = nc.dram_tensor("Wq", [DIN, DOUT], f32, kind="ExternalInput").ap()
    Wk = nc.dram_tensor("Wk", [DIN, DOUT], f32, kind="ExternalInput").ap()
    Wv = nc.dram_tensor("Wv", [DIN, DOUT], f32, kind="ExternalInput").ap()
    Woff = nc.dram_tensor("Woff", [DIN, SKV], f32, kind="ExternalInput").ap()
    out = nc.dram_tensor("out", [SQ, DOUT], f32, kind="ExternalOutput").ap()

    KI = DIN // P    # 4  din tiles
    MO = DOUT // P   # 4  dout tiles
    TQ = SQ // P     # 16 q tiles
    TK = SKV // P    # 16 kv tiles
    NQ = SQ // 512   # 4  q chunks of 512

    with tile.TileContext(nc) as tc:
        with (
            tc.tile_pool(name="dram", bufs=1, space="DRAM") as dram,
            tc.tile_pool(name="wts", bufs=1) as wts,
            tc.tile_pool(name="actT", bufs=3) as actT,
            tc.tile_pool(name="proj", bufs=1) as proj,
            tc.tile_pool(name="expp", bufs=1) as expp,
            tc.tile_pool(name="eps", bufs=3) as eps,
            tc.tile_pool(name="psum", bufs=4, space="PSUM") as psum,
            tc.tile_pool(name="pstp", bufs=2, space="PSUM") as pstp,
            tc.tile_pool(name="psrs", bufs=2, space="PSUM") as psrs,
        ):
            import concourse.mybir as mybir
            from concourse.masks import make_identity

            # ---- natural cast-loads for query/key (PE-transposed below) ----
            # All plain copies run before any XBAR transpose (Tile serializes
            # every copy<->transpose xbar-mode transition at ~10-25us each),
            # so query/key are transposed on the PE instead: natural bf16
            # cast-load, then tensor-engine transpose-mode.  This lets the PE
            # start ~15us in rather than waiting ~70us for staged casts.
            def load_nat(src, rows, tag):
                nat = actT.tile([P, rows // P, DIN], bf16, tag="actT")
                v3 = src.rearrange("(so p) d -> p so d", p=P)
                for r in range(8):
                    nc.gpsimd.dma_start(nat[:, ts(r, rows // P // 8), :],
                                        v3[:, ts(r, rows // P // 8), :])
                return nat

            qnat = load_nat(query, SQ, "q")

            # identity via NEFF-embedded constant: keeps the gpsimd queue
            # free for the SWDGE cast descgens at kernel start
            import ml_dtypes as _mld
            ident_dram = nc.inline_tensor(
                np.eye(P, dtype=_mld.bfloat16), name="ident_const"
            )
            ident = wts.tile([P, P], bf16, tag="ident")
            nc.sync.dma_start(ident[:], ident_dram.ap())
            # HAM warmup: PE transpose-mode doesn't count as PE-busy for the
            # clock gate, so burn ~3us of dummy matmuls while the first input
            # chunks are still in flight; M1 then starts at 2.4GHz.
            warm = psum.tile([P, P], f32, tag="mm")
            for i in range(28):
                nc.tensor.matmul(warm[:], lhsT=ident[:], rhs=ident[:],
                                 start=(i == 0), stop=(i == 27))
            # DoubleRow rhs needs dim-1 step % 16 == 0, so pad to [P, 2, 16]
            ones_sb = wts.tile([P, 2, 16], f8, tag="ones")
            nc.vector.memset(ones_sb[:], 1.0)
            expbias = wts.tile([P, 1], f32, tag="expbias")
            nc.vector.memset(expbias[:], -3.0)

            # ---- weights: cast-DMA straight into SBUF ----------------------
            wq_sb = wts.tile([P, KI, DOUT], f8, tag="wq")
            wk_sb = wts.tile([P, KI, DOUT], f8, tag="wk")
            wv_sb = wts.tile([P, KI, DOUT], bf16, tag="wv")
            nc.gpsimd.dma_start(wq_sb[:], Wq.rearrange("(ko p) n -> p ko n", p=P))
            nc.gpsimd.dma_start(wk_sb[:], Wk.rearrange("(ko p) n -> p ko n", p=P))
            nc.gpsimd.dma_start(wv_sb[:], Wv.rearrange("(ko p) n -> p ko n", p=P))

            knat = load_nat(key, SKV, "k")

            # ---- staged loads for value/offset/Woff (XBAR-transposed) ------
            stg_v = dram.tile([SKV, DIN], bf16, tag="stg_v")
            nc.gpsimd.dma_start(stg_v[:], value)
            stg_off = dram.tile([SQ, DIN], bf16, tag="stg_off")
            nc.gpsimd.dma_start(stg_off[:], offset)
            stg_woff = dram.tile([DIN, SKV], bf16, tag="stg_woff")
            nc.gpsimd.dma_start(stg_woff[:], Woff)

            # PE transpose: nat [128(seq), so, din] -> t [128(din), c, seq]
            def pe_transpose(nat, rows, ident):
                t = actT.tile([P, KI, rows], f8, tag="actT")
                for g in range(rows // 512):
                    for c in range(KI):
                        pt = pstp.tile([P, 512], bf16, tag="pst")
                        for j in range(4):
                            nc.tensor.transpose(
                                pt[:, ts(j, P)], nat[:, g * 4 + j, ts(c, P)],
                                ident,
                            )
                        nc.scalar.copy(t[:, c, ts(g, 512)], pt[:])
                return t

            # M1/M2: projections, transposed outputs [dout, seq]
            qpT = proj.tile([P, MO, SQ], f8, tag="qpT")
            kpT = proj.tile([P, MO, SKV], f8, tag="kpT")
            qT = pe_transpose(qnat, SQ, ident)
            kT = pe_transpose(knat, SKV, ident)
            for w_sb, xT, oT, NN in ((wq_sb, qT, qpT, NQ), (wk_sb, kT, kpT, NQ)):
                for m in range(MO):
                    for n in range(NN):
                        pt = psum.tile([P, 512], f32, tag="mm")
                        for k in range(KI // 2):
                            nc.tensor.matmul(
                                pt[:],
                                lhsT=w_sb[:, 2 * k : 2 * k + 2, ts(m, P)],
                                rhs=xT[:, 2 * k : 2 * k + 2, ts(n, 512)],
                                start=(k == 0),
                                stop=(k == KI // 2 - 1),
                                perf_mode=DR,
                            )
                        nc.vector.tensor_copy(oT[:, m, ts(n, 512)], pt[:])

            # XBAR transposes (single copy->transpose mode transition)
            vT = actT.tile([P, KI, SKV], bf16, tag="actT")
            for c in range(KI):
                nc.sync.dma_start_transpose(vT[:, c, :], stg_v[:, ts(c, P)])
            woffT = actT.tile([P, TK, DIN], bf16, tag="actT")
            for c in range(TK):
                nc.sync.dma_start_transpose(woffT[:, c, :], stg_woff[:, ts(c, P)])
            offT = actT.tile([P, KI, SQ], bf16, tag="actT")
            for c in range(KI):
                nc.sync.dma_start_transpose(offT[:, c, :], stg_off[:, ts(c, P)])

            # M4: scoresT [kv, q] -> exp(scale*x) -> bf16 SBUF
            expT = expp.tile([P, TK, SQ], f8, tag="expT")
            for mk in range(TK):
                for n in range(NQ):
                    pt = psum.tile([P, 512], f32, tag="mm")
                    for k in range(MO // 2):
                        nc.tensor.matmul(
                            pt[:],
                            lhsT=kpT[:, 2 * k : 2 * k + 2, ts(mk, P)],
                            rhs=qpT[:, 2 * k : 2 * k + 2, ts(n, 512)],
                            start=(k == 0),
                            stop=(k == MO // 2 - 1),
                            perf_mode=DR,
                        )
                    # bias -3 keeps exp outputs well inside fp8e4 range
                    # (max score*scale ~ 5.5); it cancels exactly in the
                    # rowsum normalization.
                    nc.scalar.activation(
                        expT[:, mk, ts(n, 512)],
                        pt[:],
                        mybir.ActivationFunctionType.Exp,
                        scale=SCALE,
                        bias=expbias[:],
                    )

            # M3: v_proj [kv, dout]  (after M4 in PE order: vT arrives while
            # M4 is running)
            vp = proj.tile([P, TK, DOUT], bf16, tag="vp")
            vp8 = proj.tile([P, TK, DOUT], f8, tag="vp8")
            for mk in range(TK):
                pt = psum.tile([P, 512], f32, tag="mm")
                for k in range(KI):
                    nc.tensor.matmul(
                        pt[:],
                        lhsT=vT[:, k, ts(mk, P)],
                        rhs=wv_sb[:, k, :],
                        start=(k == 0),
                        stop=(k == KI - 1),
                    )
                nc.vector.tensor_copy(vp[:, mk, :], pt[:])
                nc.vector.tensor_copy(vp8[:, mk, :], pt[:])

            # W3' = Woff @ v_proj   [din, dout]
            w3 = wts.tile([P, KI, DOUT], bf16, tag="w3")
            for m in range(KI):
                pt = psum.tile([P, 512], f32, tag="mm")
                for kk in range(TK):
                    nc.tensor.matmul(
                        pt[:],
                        lhsT=woffT[:, kk, ts(m, P)],
                        rhs=vp[:, kk, :],
                        start=(kk == 0),
                        stop=(kk == TK - 1),
                    )
                nc.vector.tensor_copy(w3[:, m, :], pt[:])

            # M5 + rowsum + M7 + epilogue, per q tile
            for mq in range(TQ):
                po = psum.tile([P, 512], f32, tag="mm")
                prs = psrs.tile([P, 1], f32, tag="rs")
                for kk in range(TK // 2):
                    nc.tensor.matmul(
                        po[:],
                        lhsT=expT[:, 2 * kk : 2 * kk + 2, ts(mq, P)],
                        rhs=vp8[:, 2 * kk : 2 * kk + 2, :],
                        start=(kk == 0),
                        stop=(kk == TK // 2 - 1),
                        perf_mode=DR,
                    )
                    nc.tensor.matmul(
                        prs[:],
                        lhsT=expT[:, 2 * kk : 2 * kk + 2, ts(mq, P)],
                        rhs=ones_sb[:, :, :1],
                        start=(kk == 0),
                        stop=(kk == TK // 2 - 1),
                        perf_mode=DR,
                    )
                poff = psum.tile([P, 512], f32, tag="mm")
                for k in range(KI):
                    nc.tensor.matmul(
                        poff[:],
                        lhsT=offT[:, k, ts(mq, P)],
                        rhs=w3[:, k, :],
                        start=(k == 0),
                        stop=(k == KI - 1),
                    )
                rc = eps.tile([P, 1], f32, tag="rc")
                nc.vector.reciprocal(rc[:], prs[:])
                tmp = eps.tile([P, 512], f32, tag="tmp")
                nc.vector.tensor_scalar_mul(tmp[:], po[:], rc[:])
                ot = eps.tile([P, 512], f32, tag="ot")
                nc.vector.tensor_add(ot[:], tmp[:], poff[:])
                nc.sync.dma_start(out[ts(mq, P), :], ot[:])

    nc.compile()
    return nc


def _get_nc():
    if "nc" not in _CACHED:
        _CACHED["nc"] = _build_bass()
    return _CACHED["nc"]


def kernel(**inputs):
    from concourse.bass_utils import run_bass_kernel_spmd

    nc = _get_nc()

    def f32c(x):
        return np.ascontiguousarray(np.asarray(x), dtype=np.float32)

    shared = {k: f32c(inputs[k]) for k in ("Wq", "Wk", "Wv", "Woff")}
    in_maps = [
        {
            "query": f32c(inputs["query"][c]),
            "key": f32c(inputs["key"][c]),
            "value": f32c(inputs["value"][c]),
            "offset": f32c(inputs["offset"][c]),
            **shared,
        }
        for c in range(N_CORES)
    ]
    res = run_bass_kernel_spmd(nc, in_maps, list(range(N_CORES)))
    return np.stack([res.results[c]["out"] for c in range(N_CORES)], axis=0)


def _install_ntff_shim():
    """The agent image's antenv lacks axon_hooks; recreate it so
    run_bass_kernel_spmd(trace=True) can reach the NTFF profiler."""
    import sys as _sys
    import types

    if "antenv.axon_hooks" in _sys.modules:
        return
    mod = types.ModuleType("antenv.axon_hooks")
    _state = {"hook": None}
    mod.set_axon_ntff_profile_hook = lambda h: _state.__setitem__("hook", h)
    mod.get_axon_ntff_profile_hook = lambda: _state["hook"]
    _sys.modules["antenv.axon_hooks"] = mod
    try:
        from trn_agent_boot.trn_boot import _ntff_profile_via_ctypes

        mod.set_axon_ntff_profile_hook(
            _ntff_profile_via_ctypes("/opt/axon/libaxon_pjrt.so")
        )
    except Exception as e:
        print(f"ntff shim: could not install profile hook: {e}", file=sys.stderr)


def run_traced(**inputs):
    """Like kernel(), but also returns (output, exec_time_ns) via NTFF trace."""
    _install_ntff_shim()
    from concourse.bass_utils import run_bass_kernel_spmd

    nc = _get_nc()

    def f32c(x):
        return np.ascontiguousarray(np.asarray(x), dtype=np.float32)

    shared = {k: f32c(inputs[k]) for k in ("Wq", "Wk", "Wv", "Woff")}
    in_maps = [
        {
            "query": f32c(inputs["query"][c]),
            "key": f32c(inputs["key"][c]),
            "value": f32c(inputs["value"][c]),
            "offset": f32c(inputs["offset"][c]),
            **shared,
        }
        for c in range(N_CORES)
    ]
    res = run_bass_kernel_spmd(nc, in_maps, list(range(N_CORES)), trace=True)
    outv = np.stack([res.results[c]["out"] for c in range(N_CORES)], axis=0)
    return outv, res


# revision 18
# speedup vs baseline: 1.0524x; 1.0075x over previous
"""Fused attention-with-offset kernel for Trainium2, 8-core data-parallel.

Problem (per batch element b, B=8 elements -> one NeuronCore each):
    q = query @ Wq                [SQ, D]
    k = key @ Wk                  [SKV, D]
    v = value @ Wv                [SKV, D]
    scores = (q @ k^T) / sqrt(D)  [SQ, SKV]
    attn = softmax(scores) + offset @ Woff
    out = attn @ v                [SQ, D]

Kernel strategy (all matmuls bf16 on PE, f32 PSUM accumulation):
  - offset path uses associativity: (offset@Woff)@v == offset@(Woff@v),
    cutting its FLOPs from 8.6 GF to 2.4 GF per core.
  - softmax computed unnormalized: expT[t,s] = exp(scale*scores[s,t]) in
    the [kv, q] orientation, row-sums via an extra N=1 matmul against a
    ones vector (lands [q, 1] per-partition), normalization fused into
    the epilogue as a per-partition tensor_scalar multiply.
  - activations enter SBUF transposed ([din, seq]) via a bf16 DRAM stage
    (SWDGE cast-DMA) + XBAR transpose-DMA.
"""

import os
import sys

import numpy as np

sys.path.insert(0, "/opt/trn_rl_repo")
sys.path.insert(0, "/opt/pypackages")

B, SQ, SKV, DIN, DOUT = 8, 2048, 2048, 512, 512
P = 128
SCALE = 1.0 / float(np.sqrt(DOUT))
N_CORES = 8

_CACHED = {}


def _build_bass():
    import concourse.bass as bass
    import concourse.tile as tile
    from concourse import bacc, mybir

    f32 = mybir.dt.float32
    bf16 = mybir.dt.bfloat16
    f8 = mybir.dt.float8e4
    DR = mybir.MatmulPerfMode.DoubleRow
    ts = bass.ts

    nc = bacc.Bacc(
        "TRN2",
        target_bir_lowering=False,
        debug=False,
        enable_asserts=True,
        num_devices=N_CORES,
    )

    query = nc.dram_tensor("query", [SQ, DIN], f32, kind="ExternalInput").ap()
    key = nc.dram_tensor("key", [SKV, DIN], f32, kind="ExternalInput").ap()
    value = nc.dram_tensor("value", [SKV, DIN], f32, kind="ExternalInput").ap()
    offset = nc.dram_tensor("offset", [SQ, DIN], f32, kind="ExternalInput").ap()
    Wq = nc.dram_tensor("Wq", [DIN, DOUT], f32, kind="ExternalInput").ap()
    Wk = nc.dram_tensor("Wk", [DIN, DOUT], f32, kind="ExternalInput").ap()
    Wv = nc.dram_tensor("Wv", [DIN, DOUT], f32, kind="ExternalInput").ap()
    Woff = nc.dram_tensor("Woff", [DIN, SKV], f32, kind="ExternalInput").ap()
    out = nc.dram_tensor("out", [SQ, DOUT], f32, kind="ExternalOutput").ap()

    KI = DIN // P    # 4  din tiles
    MO = DOUT // P   # 4  dout tiles
    TQ = SQ // P     # 16 q tiles
    TK = SKV // P    # 16 kv tiles
    NQ = SQ // 512   # 4  q chunks of 512

    with tile.TileContext(nc) as tc:
        with (
            tc.tile_pool(name="dram", bufs=1, space="DRAM") as dram,
            tc.tile_pool(name="wts", bufs=1) as wts,
            tc.tile_pool(name="actT", bufs=3) as actT,
            tc.tile_pool(name="proj", bufs=1) as proj,
            tc.tile_pool(name="expp", bufs=1) as expp,
            tc.tile_pool(name="eps", bufs=3) as eps,
            tc.tile_pool(name="psum", bufs=4, space="PSUM") as psum,
            tc.tile_pool(name="pstp", bufs=2, space="PSUM") as pstp,
            tc.tile_pool(name="psrs", bufs=2, space="PSUM") as psrs,
        ):
            import concourse.mybir as mybir
            from concourse.masks import make_identity

            # ---- natural cast-loads for query/key (PE-transposed below) ----
            # All plain copies run before any XBAR transpose (Tile serializes
            # every copy<->transpose xbar-mode transition at ~10-25us each),
            # so query/key are transposed on the PE instead: natural bf16
            # cast-load, then tensor-engine transpose-mode.  This lets the PE
            # start ~15us in rather than waiting ~70us for staged casts.
            def load_nat(src, rows, tag):
                nat = actT.tile([P, rows // P, DIN], bf16, tag="actT")
                v3 = src.rearrange("(so p) d -> p so d", p=P)
                for r in range(8):
                    nc.gpsimd.dma_start(nat[:, ts(r, rows // P // 8), :],
                                        v3[:, ts(r, rows // P // 8), :])
                return nat

            qnat = load_nat(query, SQ, "q")

            # identity via NEFF-embedded constant: keeps the gpsimd queue
            # free for the SWDGE cast descgens at kernel start
            import ml_dtypes as _mld
            ident_dram = nc.inline_tensor(
                np.eye(P, dtype=_mld.bfloat16), name="ident_const"
            )
            ident = wts.tile([P, P], bf16, tag="ident")
            nc.sync.dma_start(ident[:], ident_dram.ap())
            # DoubleRow rhs needs dim-1 step % 16 == 0, so pad to [P, 2, 16]
            ones_sb = wts.tile([P, 2, 16], f8, tag="ones")
            nc.vector.memset(ones_sb[:], 1.0)
            expbias = wts.tile([P, 1], f32, tag="expbias")
            nc.vector.memset(expbias[:], -3.0)

            # ---- weights: cast-DMA straight into SBUF ----------------------
            wq_sb = wts.tile([P, KI, DOUT], f8, tag="wq")
            wk_sb = wts.tile([P, KI, DOUT], f8, tag="wk")
            wv_sb = wts.tile([P, KI, DOUT], bf16, tag="wv")
            nc.gpsimd.dma_start(wq_sb[:], Wq.rearrange("(ko p) n -> p ko n", p=P))
            nc.gpsimd.dma_start(wk_sb[:], Wk.rearrange("(ko p) n -> p ko n", p=P))
            nc.gpsimd.dma_start(wv_sb[:], Wv.rearrange("(ko p) n -> p ko n", p=P))

            knat = load_nat(key, SKV, "k")

            # ---- staged loads for value/offset/Woff (XBAR-transposed) ------
            stg_v = dram.tile([SKV, DIN], bf16, tag="stg_v")
            nc.gpsimd.dma_start(stg_v[:], value)
            stg_off = dram.tile([SQ, DIN], bf16, tag="stg_off")
            nc.gpsimd.dma_start(stg_off[:], offset)
            stg_woff = dram.tile([DIN, SKV], bf16, tag="stg_woff")
            nc.gpsimd.dma_start(stg_woff[:], Woff)

            # PE transpose: nat [128(seq), so, din] -> t [128(din), c, seq]
            def pe_transpose(nat, rows, ident):
                t = actT.tile([P, KI, rows], f8, tag="actT")
                for g in range(rows // 512):
                    for c in range(KI):
                        pt = pstp.tile([P, 512], bf16, tag="pst")
                        for j in range(4):
                            nc.tensor.transpose(
                                pt[:, ts(j, P)], nat[:, g * 4 + j, ts(c, P)],
                                ident,
                            )
                        nc.scalar.copy(t[:, c, ts(g, 512)], pt[:])
                return t

            # M1/M2: projections, transposed outputs [dout, seq]
            qpT = proj.tile([P, MO, SQ], f8, tag="qpT")
            kpT = proj.tile([P, MO, SKV], f8, tag="kpT")
            qT = pe_transpose(qnat, SQ, ident)
            kT = pe_transpose(knat, SKV, ident)
            for w_sb, xT, oT, NN in ((wq_sb, qT, qpT, NQ), (wk_sb, kT, kpT, NQ)):
                for m in range(MO):
                    for n in range(NN):
                        pt = psum.tile([P, 512], f32, tag="mm")
                        for k in range(KI // 2):
                            nc.tensor.matmul(
                                pt[:],
                                lhsT=w_sb[:, 2 * k : 2 * k + 2, ts(m, P)],
                                rhs=xT[:, 2 * k : 2 * k + 2, ts(n, 512)],
                                start=(k == 0),
                                stop=(k == KI // 2 - 1),
                                perf_mode=DR,
                            )
                        nc.vector.tensor_copy(oT[:, m, ts(n, 512)], pt[:])

            # XBAR transposes (single copy->transpose mode transition)
            vT = actT.tile([P, KI, SKV], bf16, tag="actT")
            for c in range(KI):
                nc.sync.dma_start_transpose(vT[:, c, :], stg_v[:, ts(c, P)])
            woffT = actT.tile([P, TK, DIN], bf16, tag="actT")
            for c in range(TK):
                nc.sync.dma_start_transpose(woffT[:, c, :], stg_woff[:, ts(c, P)])
            offT = actT.tile([P, KI, SQ], bf16, tag="actT")
            for c in range(KI):
                nc.sync.dma_start_transpose(offT[:, c, :], stg_off[:, ts(c, P)])

            # M4: scoresT [kv, q] -> exp(scale*x) -> bf16 SBUF
            expT = expp.tile([P, TK, SQ], f8, tag="expT")
            for mk in range(TK):
                for n in range(NQ):
                    pt = psum.tile([P, 512], f32, tag="mm")
                    for k in range(MO // 2):
                        nc.tensor.matmul(
                            pt[:],
                            lhsT=kpT[:, 2 * k : 2 * k + 2, ts(mk, P)],
                            rhs=qpT[:, 2 * k : 2 * k + 2, ts(n, 512)],
                            start=(k == 0),
                            stop=(k == MO // 2 - 1),
                            perf_mode=DR,
                        )
                    # bias -3 keeps exp outputs well inside fp8e4 range
                    # (max score*scale ~ 5.5); it cancels exactly in the
                    # rowsum normalization.
                    nc.scalar.activation(
                        expT[:, mk, ts(n, 512)],
                        pt[:],
                        mybir.ActivationFunctionType.Exp,
                        scale=SCALE,
                        bias=expbias[:],
                    )

            # M3: v_proj [kv, dout]  (after M4 in PE order: vT arrives while
            # M4 is running)
            vp = proj.tile([P, TK, DOUT], bf16, tag="vp")
            vp8 = proj.tile([P, TK, DOUT], f8, tag="vp8")
            for mk in range(TK):
                pt = psum.tile([P, 512], f32, tag="mm")
                for k in range(KI):
                    nc.tensor.matmul(
                        pt[:],
                        lhsT=vT[:, k, ts(mk, P)],
                        rhs=wv_sb[:, k, :],
                        start=(k == 0),
                        stop=(k == KI - 1),
                    )
                nc.vector.tensor_copy(vp[:, mk, :], pt[:])
                nc.vector.tensor_copy(vp8[:, mk, :], pt[:])

            # W3' = Woff @ v_proj   [din, dout]
            w3 = wts.tile([P, KI, DOUT], bf16, tag="w3")
            for m in range(KI):
                pt = psum.tile([P, 512], f32, tag="mm")
                for kk in range(TK):
                    nc.tensor.matmul(
                        pt[:],
                        lhsT=woffT[:, kk, ts(m, P)],
                        rhs=vp[:, kk, :],
                        start=(kk == 0),
                        stop=(kk == TK - 1),
                    )
                nc.vector.tensor_copy(w3[:, m, :], pt[:])

            # M5 + rowsum + M7 + epilogue, per q tile
            for mq in range(TQ):
                po = psum.tile([P, 512], f32, tag="mm")
                prs = psrs.tile([P, 1], f32, tag="rs")
                for kk in range(TK // 2):
                    nc.tensor.matmul(
                        po[:],
                        lhsT=expT[:, 2 * kk : 2 * kk + 2, ts(mq, P)],
                        rhs=vp8[:, 2 * kk : 2 * kk + 2, :],
                        start=(kk == 0),
                        stop=(kk == TK // 2 - 1),
                        perf_mode=DR,
                    )
                    nc.tensor.matmul(
                        prs[:],
                        lhsT=expT[:, 2 * kk : 2 * kk + 2, ts(mq, P)],
                        rhs=ones_sb[:, :, :1],
                        start=(kk == 0),
                        stop=(kk == TK // 2 - 1),
                        perf_mode=DR,
                    )
                poff = psum.tile([P, 512], f32, tag="mm")
                for k in range(KI):
                    nc.tensor.matmul(
                        poff[:],
                        lhsT=offT[:, k, ts(mq, P)],
                        rhs=w3[:, k, :],
                        start=(k == 0),
                        stop=(k == KI - 1),
                    )
                rc = eps.tile([P, 1], f32, tag="rc")
                nc.vector.reciprocal(rc[:], prs[:])
                tmp = eps.tile([P, 512], f32, tag="tmp")
                nc.vector.tensor_scalar_mul(tmp[:], po[:], rc[:])
                ot = eps.tile([P, 512], f32, tag="ot")
                nc.vector.tensor_add(ot[:], tmp[:], poff[:])
                nc.sync.dma_start(out[ts(mq, P), :], ot[:])

    nc.compile()
    return nc


def _get_nc():
    if "nc" not in _CACHED:
        _CACHED["nc"] = _build_bass()
    return _CACHED["nc"]


def kernel(**inputs):
    from concourse.bass_utils import run_bass_kernel_spmd

    nc = _get_nc()

    def f32c(x):
        return np.ascontiguousarray(np.asarray(x), dtype=np.float32)

    shared = {k: f32c(inputs[k]) for k in ("Wq", "Wk", "Wv", "Woff")}
    in_maps = [
        {
            "query": f32c(inputs["query"][c]),
            "key": f32c(inputs["key"][c]),
            "value": f32c(inputs["value"][c]),
            "offset": f32c(inputs["offset"][c]),
            **shared,
        }
        for c in range(N_CORES)
    ]
    res = run_bass_kernel_spmd(nc, in_maps, list(range(N_CORES)))
    return np.stack([res.results[c]["out"] for c in range(N_CORES)], axis=0)


def _install_ntff_shim():
    """The agent image's antenv lacks axon_hooks; recreate it so
    run_bass_kernel_spmd(trace=True) can reach the NTFF profiler."""
    import sys as _sys
    import types

    if "antenv.axon_hooks" in _sys.modules:
        return
    mod = types.ModuleType("antenv.axon_hooks")
    _state = {"hook": None}
    mod.set_axon_ntff_profile_hook = lambda h: _state.__setitem__("hook", h)
    mod.get_axon_ntff_profile_hook = lambda: _state["hook"]
    _sys.modules["antenv.axon_hooks"] = mod
    try:
        from trn_agent_boot.trn_boot import _ntff_profile_via_ctypes

        mod.set_axon_ntff_profile_hook(
            _ntff_profile_via_ctypes("/opt/axon/libaxon_pjrt.so")
        )
    except Exception as e:
        print(f"ntff shim: could not install profile hook: {e}", file=sys.stderr)


def run_traced(**inputs):
    """Like kernel(), but also returns (output, exec_time_ns) via NTFF trace."""
    _install_ntff_shim()
    from concourse.bass_utils import run_bass_kernel_spmd

    nc = _get_nc()

    def f32c(x):
        return np.ascontiguousarray(np.asarray(x), dtype=np.float32)

    shared = {k: f32c(inputs[k]) for k in ("Wq", "Wk", "Wv", "Woff")}
    in_maps = [
        {
            "query": f32c(inputs["query"][c]),
            "key": f32c(inputs["key"][c]),
            "value": f32c(inputs["value"][c]),
            "offset": f32c(inputs["offset"][c]),
            **shared,
        }
        for c in range(N_CORES)
    ]
    res = run_bass_kernel_spmd(nc, in_maps, list(range(N_CORES)), trace=True)
    outv = np.stack([res.results[c]["out"] for c in range(N_CORES)], axis=0)
    return outv, res
